# revision 3
# baseline (speedup 1.0000x reference)
"""NNUE embedding-lookup + tiny-MLP kernel for Trainium2 (8 NeuronCores), v3.

Data-parallel over batch: each core handles 2048 of the 16384 positions.

v3 design vs v2 (182788 ns):
  - W1 folded into the table (NNUE feature-transformer trick): the host
    precomputes P = emb @ w1.T [49152, 32] f32 once per weight set; the sum
    over active features commutes with the linear layer, so the kernel
    gathers 32-float rows instead of 256-dim bf16 rows. Rows are padded to
    64 f32 (= 256 B, the dma_gather minimum element) with zeros.
    Probe-measured: 256 B gather packets cost ~20 ns wall vs ~35 ns for the
    512 B packets v2 used -> gather phase ~84 us instead of ~131 us.
  - Two gather calls per 128-batch t-block (2048 idx each, j = f_local*128+b;
    4096-idx calls overflowed the per-queue SWDGE descriptor ring and
    serialized the Q7 engine), 32 calls on 4 SWDGE queues.
  - No warmup call: queue 0's SWDGE path generates descriptors ~8x slower
    than queues 1-3, so the rotation starts at queue 1 and queue 0's slow
    generation overlaps three fast calls each round.
  - idx uploaded in 4 slices (first covers t-block 0) on the sync/scalar
    HWDGE rings -> first real gather is not gated on the full 1 MB upload.
  - Feature-sum: 5-level f32 tensor_tensor tree on DVE (free sizes
    1024+512+256+128+64 per t-block ~ 2.1 us; DVE total ~45 us, hidden
    under the gather).
  - TensorE: one f32 transpose (x [128,64] -> PSUM [64,128]) + the
    32->32->1 MLP per t-block; ScalarE applies bias+ReLU/Tanh from PSUM.
"""

import numpy as np

INPUT_DIM = 49152
E = 256
D = 32                         # MLP hidden / projected row dim
DP = 64                        # padded projected row (f32) = 256 B
BATCH = 16384
F = 32
N_CORES = 8
B_CORE = BATCH // N_CORES      # 2048
BIAS = 16384                   # index bias for int16 gather
CHUNK = 2048                   # idx per gather call (16 features x 128 b)
CW = CHUNK // 16               # 128 idx cols per call
NT = B_CORE // 128             # 16 t-blocks
NCALL = 2 * NT                 # 32 gather calls
IDX_COLS = NCALL * CW          # 4096

_nc_cache = None


def _build():
    import concourse.bacc as bacc
    import concourse.mybir as mybir
    import concourse.tile as tile

    f32 = mybir.dt.float32
    i16 = mybir.dt.int16
    AF = mybir.ActivationFunctionType
    ADD = mybir.AluOpType.add

    nc = bacc.Bacc(
        None,
        target_bir_lowering=False,
        debug=False,
        num_swdge_queues=4,
        dynamic_dma_scratch_size=49152,
    )
    emb32 = nc.dram_tensor("emb32", [INPUT_DIM, DP], f32, kind="ExternalInput")
    idx = nc.dram_tensor("idx", [128, IDX_COLS], i16, kind="ExternalInput")
    identf = nc.dram_tensor("identf", [128, 128], f32, kind="ExternalInput")
    b1 = nc.dram_tensor("b1", [D, 1], f32, kind="ExternalInput")
    w2l = nc.dram_tensor("w2l", [D, D], f32, kind="ExternalInput")
    b2 = nc.dram_tensor("b2", [D, 1], f32, kind="ExternalInput")
    w3l = nc.dram_tensor("w3l", [D, 1], f32, kind="ExternalInput")
    b3 = nc.dram_tensor("b3", [1, 1], f32, kind="ExternalInput")
    out = nc.dram_tensor("out", [1, B_CORE], f32, kind="ExternalOutput")

    with tile.TileContext(nc) as tc:
        with (
            tc.tile_pool(name="const", bufs=1) as cpool,
            tc.tile_pool(name="g", bufs=8) as gpool,
            tc.tile_pool(name="l1", bufs=2) as l1pool,
            tc.tile_pool(name="l2", bufs=2) as l2pool,
            tc.tile_pool(name="l3", bufs=2) as l3pool,
            tc.tile_pool(name="l4", bufs=2) as l4pool,
            tc.tile_pool(name="x", bufs=2) as xpool,
            tc.tile_pool(name="h", bufs=4) as hpool,
            tc.tile_pool(name="xtp", bufs=2, space="PSUM") as xtppool,
            tc.tile_pool(name="mp", bufs=2, space="PSUM") as mppool,
        ):
            idx_t = cpool.tile([128, IDX_COLS], i16)
            slices = [(0, CW), (CW, 1280), (CW + 1280, 1280), (CW + 2560, IDX_COLS - CW - 2560)]
            for k, (lo, n) in enumerate(slices):
                eng = nc.sync if k % 2 == 0 else nc.scalar
                eng.dma_start(idx_t[:, lo : lo + n], idx[:, lo : lo + n])
            identf_t = cpool.tile([128, 128], f32)
            nc.scalar.dma_start(identf_t[:], identf[:])
            b1_t = cpool.tile([D, 1], f32)
            nc.sync.dma_start(b1_t[:], b1[:])
            w2l_t = cpool.tile([D, D], f32)
            nc.sync.dma_start(w2l_t[:], w2l[:])
            b2_t = cpool.tile([D, 1], f32)
            nc.sync.dma_start(b2_t[:], b2[:])
            w3l_t = cpool.tile([D, 1], f32)
            nc.sync.dma_start(w3l_t[:], w3l[:])
            b3_t = cpool.tile([1, 1], f32)
            nc.sync.dma_start(b3_t[:], b3[:])
            out_t = cpool.tile([1, B_CORE], f32)

            nreg = nc.gpsimd.to_reg(CHUNK)

            # Queues rotate 1,2,3,0. Measured pacing: each queue completes
            # one call per ~16.5 us end-to-end (desc-gen ~1.5 us + its 2064
            # packets draining at 1/4 of the 16-engine aggregate ~12.2 us +
            # sem/ring handshake ~3 us), so 4 queues x 8 calls x 16.5/4
            # ~= 132 us paces the gather phase. Calls must stay uniform
            # (identical static config, no padding, no interleaved Pool
            # DMAs) -- any deviation reloads the Q7 gather ucode at ~16 us
            # per call.
            qn = 1
            for t in range(NT):
                l1s = []
                for half in range(2):
                    g = gpool.tile([128, 16, DP], f32, tag="g")
                    col = (2 * t + half) * CW
                    nc.gpsimd.dma_gather(
                        g[:], emb32[BIAS:, :], idx_t[:, col : col + CW],
                        CHUNK, nreg, DP, single_packet=False, queue_num=qn % 4,
                    )
                    qn += 1
                    l1 = l1pool.tile([128, 8, DP], f32, tag=f"l1{half}")
                    nc.vector.tensor_tensor(
                        out=l1[:], in0=g[:, 0:8, :], in1=g[:, 8:16, :], op=ADD
                    )
                    l1s.append(l1)
                l2 = l2pool.tile([128, 8, DP], f32, tag="l2")
                nc.vector.tensor_tensor(
                    out=l2[:], in0=l1s[0][:], in1=l1s[1][:], op=ADD
                )
                l3 = l3pool.tile([128, 4, DP], f32, tag="l3")
                nc.vector.tensor_tensor(
                    out=l3[:], in0=l2[:, 0:4, :], in1=l2[:, 4:8, :], op=ADD
                )
                l4 = l4pool.tile([128, 2, DP], f32, tag="l4")
                nc.vector.tensor_tensor(
                    out=l4[:], in0=l3[:, 0:2, :], in1=l3[:, 2:4, :], op=ADD
                )
                x = xpool.tile([128, DP], f32, tag="x")
                nc.vector.tensor_tensor(
                    out=x[:], in0=l4[:, 0, :], in1=l4[:, 1, :], op=ADD
                )
                xt = xtppool.tile([DP, 128], f32, tag="xtp")
                nc.tensor.transpose(xt[:], x[:], identf_t[:])
                h1 = hpool.tile([D, 128], f32, tag="h1")
                nc.scalar.activation(h1[:], xt[0:D, :], AF.Relu, bias=b1_t[:])
                h2p = mppool.tile([D, 128], f32, tag="mp")
                nc.tensor.matmul(h2p[:], lhsT=w2l_t[:], rhs=h1[:], start=True, stop=True)
                h2 = hpool.tile([D, 128], f32, tag="h2")
                nc.scalar.activation(h2[:], h2p[:], AF.Relu, bias=b2_t[:])
                yp = mppool.tile([1, 128], f32, tag="yp")
                nc.tensor.matmul(yp[:], lhsT=w3l_t[:], rhs=h2[:], start=True, stop=True)
                nc.scalar.activation(
                    out_t[:, 128 * t : 128 * (t + 1)], yp[:], AF.Tanh, bias=b3_t[:]
                )
                if t == NT // 2 - 1:
                    nc.sync.dma_start(
                        out[:, : B_CORE // 2], out_t[:, : B_CORE // 2]
                    )
                elif t == NT - 1:
                    nc.sync.dma_start(
                        out[:, B_CORE // 2 :], out_t[:, B_CORE // 2 :]
                    )

    nc.compile()
    return nc


def _get_nc():
    global _nc_cache
    if _nc_cache is None:
        _nc_cache = _build()
    return _nc_cache


def _prep_indices(shard: np.ndarray) -> np.ndarray:
    """[F, B_CORE] int -> [128, IDX_COLS] int16 device layout.

    Per t-block one call of 4096 indices, j = f*128 + b wrapped [16, CW]
    column-major (lay[j%16, j//16] = idx_j) and replicated across the 8 Q7
    core groups. The SWDGE trims trailing negative (biased) indices, so the
    features of batch 127 are permuted to put a non-negative index at
    j=4095 (f=31, b=127).
    """
    arr = np.asarray(shard).reshape(F, NT, 128).astype(np.int64) - BIAS
    outa = np.zeros((128, IDX_COLS), np.int16)
    for t in range(NT):
        # call ends are (f=15, b=127) and (f=31, b=127): permute batch 127's
        # features so both are non-negative (trailing-negative trim no-op)
        feats = arr[:, t, 127].copy()
        nn = [i for i in range(F) if feats[i] >= 0]
        assert len(nn) >= 2, "batch 127 lacks non-negative features"
        perm = list(range(F))
        for slot in (15, 31):
            if feats[perm[slot]] < 0:
                for j in nn:
                    pj = perm.index(j)
                    if pj not in (15, 31):
                        perm[slot], perm[pj] = perm[pj], perm[slot]
                        break
        arr[:, t, 127] = feats[perm]
        for half in range(2):
            flat = arr[16 * half : 16 * half + 16, t, :].reshape(-1)
            lay = flat.reshape(CW, 16).T         # [16, CW]
            col = (2 * t + half) * CW
            outa[:, col : col + CW] = np.tile(lay, (8, 1))
    return outa


def build_in_maps(inputs: dict) -> list[dict]:
    indices = np.asarray(inputs["indices"])
    emb = np.asarray(inputs["emb"], dtype=np.float32)
    w1 = np.asarray(inputs["w1"], dtype=np.float32)
    b1 = np.asarray(inputs["b1"], dtype=np.float32)
    w2 = np.asarray(inputs["w2"], dtype=np.float32)
    b2 = np.asarray(inputs["b2"], dtype=np.float32)
    w3 = np.asarray(inputs["w3"], dtype=np.float32)
    b3 = np.asarray(inputs["b3"], dtype=np.float32)

    proj = np.zeros((INPUT_DIM, DP), np.float32)
    proj[:, :D] = emb @ w1.T
    common = {
        "emb32": proj,
        "identf": np.eye(128, dtype=np.float32),
        "b1": b1.reshape(D, 1),
        "w2l": np.ascontiguousarray(w2.T),
        "b2": b2.reshape(D, 1),
        "w3l": np.ascontiguousarray(w3.T),
        "b3": b3.reshape(1, 1),
    }
    in_maps = []
    for c in range(N_CORES):
        shard = indices[:, c * B_CORE : (c + 1) * B_CORE]
        in_maps.append({**common, "idx": _prep_indices(shard)})
    return in_maps


def kernel(**inputs) -> np.ndarray:
    from concourse.bass_utils import run_bass_kernel_spmd

    in_maps = build_in_maps(inputs)
    nc = _get_nc()
    res = run_bass_kernel_spmd(nc, in_maps, core_ids=list(range(N_CORES)))
    ys = [np.asarray(res.results[c]["out"]).reshape(B_CORE) for c in range(N_CORES)]
    return np.concatenate(ys).reshape(BATCH, 1).astype(np.float32)
